# revision 1
# baseline (speedup 1.0000x reference)
"""Multi-head attention (B=4, S=2048, D=1024, H=16, HD=64) on 8 TRN2 cores.

Sharding: batch (4) x head-halves (2) -> 8 cores; core c handles batch c//2
and heads [8*(c%2), 8*(c%2)+8) (512 columns of every projection). No
cross-core communication.

Per-core kernel (Tile framework, bf16 matmuls / fp32 accumulation):
  - X (Q/K/V batch slices) DMA'd in fp32, cast to bf16 on GpSimd, transposed
    to d-major layout with the xbar transpose DMA.
  - Projections on TensorE: qT/kT produced feature-major [head-pair cols, toks]
    (exactly the layout attention wants), V produced token-major with a ones
    column interleaved per head ([v_h | 1] is the PV stationary operand, so
    softmax denominators fall out of the PV matmul for free).
  - scores^T = kT.T @ qT per 128-token k-chunk, two heads packed in the
    128x128 PE array via row tiling (each head only contracts over 64 rows).
  - softmax: exp on ScalarE straight out of PSUM (scale=1/8 folded in, no
    max-subtraction -- scores are O(1) by construction), bf16 P^T to SBUF.
  - out^T[hd+1, q] accumulates over k-chunks in PSUM; row 64 is the denom.
  - finalize: PE-transpose to token-major, multiply by reciprocal denom, DMA.
"""

import numpy as np

import concourse.bass as bass
import concourse.tile as tile
from concourse import mybir
from concourse.masks import make_identity

B, S, D_IN, D_MODEL, H = 4, 2048, 1024, 1024, 16
HD = D_MODEL // H  # 64
N_CORES = 8
COLS = 512  # per-core projection columns (8 heads)
NPAIR = 4  # head pairs per core
NKC = S // 128  # 16 k-chunks
NDC = D_IN // 128  # 8 d_in chunks
QB = 512  # q block
NQB = S // QB  # 4

F32 = mybir.dt.float32
BF16 = mybir.dt.bfloat16
EXP = mybir.ActivationFunctionType.Exp


def _fixup_multi_waits(nc):
    """Split >cap sync waits per instruction into preceding same-engine NoOps.

    This walrus build rejects more than 1 sync wait command per instruction
    (2 for EventSemaphore); Tile's drain/backedge paths can attach one wait
    per live semaphore to a single Drain.
    """
    for fn in nc.m.functions:
        for block in fn.blocks:
            insts = block.instructions
            i = 0
            while i < len(insts):
                inst = insts[i]
                si = inst.sync_info
                cap = 2 if isinstance(inst, mybir.InstEventSemaphore) else 1
                if si is not None and len(si.on_wait) > cap:
                    waits = list(si.on_wait)
                    keep, extra = waits[:cap], waits[cap:]
                    inst.sync_info = mybir.SyncInfo(
                        on_wait=keep, on_update=list(si.on_update)
                    )
                    nops = [
                        mybir.InstNoOp(
                            name=f"{inst.name}_xwait{j}",
                            engine=inst.engine,
                            bass_nofuse=True,
                            sync_info=mybir.SyncInfo(on_wait=[w], on_update=[]),
                        )
                        for j, w in enumerate(extra)
                    ]
                    insts[i:i] = nops
                    i += len(nops)
                i += 1


class _TC(tile.TileContext):
    def __exit__(self, *args):
        ret = super().__exit__(*args)
        _fixup_multi_waits(self.nc)
        return ret


def build_core_program(fixup_waits: bool = True, time_reps: int = 1, prologue_only: bool = False, ablate: str = '') -> bass.Bass:
    tc_cls = _TC if fixup_waits else tile.TileContext
    nc = bass.Bass()
    xq = nc.dram_tensor("xq", [S, D_IN], F32, kind="ExternalInput")
    xk = nc.dram_tensor("xk", [S, D_IN], F32, kind="ExternalInput")
    xv = nc.dram_tensor("xv", [S, D_IN], F32, kind="ExternalInput")
    wq = nc.dram_tensor("wq", [D_IN, COLS], F32, kind="ExternalInput")
    wk = nc.dram_tensor("wk", [D_IN, COLS], F32, kind="ExternalInput")
    wv = nc.dram_tensor("wv", [D_IN, COLS], F32, kind="ExternalInput")
    bqp = nc.dram_tensor("bqp", [128, NPAIR], F32, kind="ExternalInput")
    bkp = nc.dram_tensor("bkp", [128, NPAIR], F32, kind="ExternalInput")
    bvb = nc.dram_tensor("bvb", [128, COLS], F32, kind="ExternalInput")
    out = nc.dram_tensor("out", [S, COLS], F32, kind="ExternalOutput")

    from contextlib import ExitStack

    with tc_cls(nc) as tc:
        with ExitStack() as ctx:
            ec = ctx.enter_context
            cpool = ec(tc.tile_pool(name="const", bufs=1))
            wpool = ec(tc.tile_pool(name="wsb", bufs=1))
            xstage_pool = ec(tc.tile_pool(name="xstage", bufs=5))
            xbf_pool = ec(tc.tile_pool(name="xbf", bufs=5))
            xtq_pool = ec(tc.tile_pool(name="xtq", bufs=2))
            xtkv_pool = ec(tc.tile_pool(name="xtkv", bufs=2))
            ktv_pool = ec(tc.tile_pool(name="ktv", bufs=1))
            qt_pool = ec(tc.tile_pool(name="qt", bufs=1))
            pt_pool = ec(tc.tile_pool(name="pt", bufs=3))
            outt_pool = ec(tc.tile_pool(name="outt", bufs=2))
            small_pool = ec(tc.tile_pool(name="small", bufs=2))
            ob_pool = ec(tc.tile_pool(name="ob", bufs=2))
            sc_ps = ec(tc.tile_pool(name="psc", bufs=2, space="PSUM"))
            proj_ps = ec(tc.tile_pool(name="pproj", bufs=2, space="PSUM"))
            pv_ps = ec(tc.tile_pool(name="ppv", bufs=2, space="PSUM"))
                        # --- constants ---
            idn = cpool.tile([128, 128], F32)
            make_identity(nc, idn[:])
            bqp_sb = cpool.tile([128, NPAIR], F32, tag="bqp")
            bkp_sb = cpool.tile([128, NPAIR], F32, tag="bkp")
            bvb_sb = cpool.tile([128, COLS], F32, tag="bvb")
            nc.sync.dma_start(bqp_sb[:], bqp[:])
            nc.sync.dma_start(bkp_sb[:], bkp[:])
            nc.sync.dma_start(bvb_sb[:], bvb[:])

            for _rep in range(time_reps):
                # --- weights: SWDGE cast-DMA fp32->bf16 ---
                w_sb = {}
                for name, wd in (("q", wq), ("k", wk), ("v", wv)):
                    wsb = wpool.tile([128, NDC, COLS], BF16, tag=f"w{name}")
                    nc.gpsimd.dma_start(
                        wsb[:], wd.rearrange("(c p) n -> p c n", p=128)
                    )
                    w_sb[name] = wsb

                def load_xt_block(xdram, dest, tc0, ntc):
                    # dest[:, dc, (tc-tc0)*128 + t] = x[tc*128 + t, dc*128 + dp]
                    # fp32 loads on the SP HWDGE ring, casts split across
                    # GpSimd/DVE, then grouped xbar transposes (also SP ring).
                    xbs = []
                    for tci in range(tc0, tc0 + ntc):
                        st = xstage_pool.tile([128, D_IN], F32, tag="xst")
                        nc.sync.dma_start(st[:], xdram[tci * 128 : (tci + 1) * 128, :])
                        xb = xbf_pool.tile([128, D_IN], BF16, tag="xbf")
                        eng = nc.gpsimd if tci % 2 == 0 else nc.vector
                        eng.tensor_copy(xb[:], st[:])
                        xbs.append(xb)
                    for i, xb in enumerate(xbs):
                        o = i * 128
                        nc.sync.dma_start(
                            dest[:, :, o : o + 128], xb[:], transpose=True
                        )

                # --- block-interleaved ingest of q/k/v (per-block tiles;
                # fine-grained deps so attention starts after block 0) ---
                qT_all, kT_blk, v_blk = [], [], []
                for blk in range(4):
                    xtqb = xtq_pool.tile([128, NDC, QB], BF16, tag="xtq")
                    load_xt_block(xq, xtqb, blk * 4, 4)
                    qT_blk = qt_pool.tile([128, NPAIR, QB], BF16, tag=f"qt{blk}")
                    qT_all.append(qT_blk)
                    for p in range(NPAIR):
                        ps = proj_ps.tile([128, 512], F32, tag="proj")
                        for dc in range(NDC):
                            nc.tensor.matmul(
                                ps[:],
                                w_sb["q"][:, dc, p * 128 : (p + 1) * 128],
                                xtqb[:, dc, :],
                                start=(dc == 0),
                                stop=(dc == NDC - 1),
                            )
                        nc.vector.tensor_scalar_add(
                            qT_blk[:, p, :], ps[:], bqp_sb[:, p : p + 1]
                        )
                    xtk = xtkv_pool.tile([128, NDC, 512], BF16, tag="xtkv")
                    load_xt_block(xk, xtk, blk * 4, 4)
                    kT = ktv_pool.tile([128, NPAIR, 512], BF16, tag=f"kT{blk}")
                    kT_blk.append(kT)
                    for p in range(NPAIR):
                        ps = proj_ps.tile([128, 512], F32, tag="proj")
                        for dc in range(NDC):
                            nc.tensor.matmul(
                                ps[:],
                                w_sb["k"][:, dc, p * 128 : (p + 1) * 128],
                                xtk[:, dc, :],
                                start=(dc == 0),
                                stop=(dc == NDC - 1),
                            )
                        nc.vector.tensor_scalar_add(
                            kT[:, p, :], ps[:], bkp_sb[:, p : p + 1]
                        )
                    xtv = xtkv_pool.tile([128, NDC, 512], BF16, tag="xtkv")
                    load_xt_block(xv, xtv, blk * 4, 4)
                    vb = ktv_pool.tile([128, 4, 8, HD + 1], BF16, tag=f"v{blk}")
                    v_blk.append(vb)
                    nc.gpsimd.memset(vb[:, :, :, HD : HD + 1], 1.0)
                    for tci in range(4):
                        ps = proj_ps.tile([128, 512], F32, tag="proj")
                        for dc in range(NDC):
                            nc.tensor.matmul(
                                ps[:],
                                xtv[:, dc, tci * 128 : (tci + 1) * 128],
                                w_sb["v"][:, dc, :],
                                start=(dc == 0),
                                stop=(dc == NDC - 1),
                            )
                        nc.vector.tensor_add(
                            vb[:, tci, :, 0:HD],
                            ps[:].rearrange("p (h d) -> p h d", h=8),
                            bvb_sb[:].rearrange("p (h d) -> p h d", h=8),
                        )

                GROUPS = [(0, 2), (2, 2), (4, 2), (6, 2), (8, 2), (10, 2), (12, 2), (14, 2)]

                def scores_grp(j, p, g, qT_blk, pTa, pTb):
                    kc0, glen = GROUPS[g]
                    scA = sc_ps.tile([128, glen * 512], F32, tag="sc")
                    scB = sc_ps.tile([128, glen * 512], F32, tag="sc")
                    for u in range(glen):
                        kc = kc0 + u
                        nc.tensor.matmul(
                            scA[:, u * 512 : (u + 1) * 512],
                            kT_blk[kc // 4][0:64, p, (kc % 4) * 128 : (kc % 4 + 1) * 128],
                            qT_blk[0:64, p, :],
                            start=True,
                            stop=True,
                        )
                        nc.tensor.matmul(
                            scB[:, u * 512 : (u + 1) * 512],
                            kT_blk[kc // 4][64:128, p, (kc % 4) * 128 : (kc % 4 + 1) * 128],
                            qT_blk[64:128, p, :],
                            tile_position=(64, 0),
                            start=True,
                            stop=True,
                        )
                    ew = glen * 512 // 8 if ablate == "exp_narrow" else glen * 512
                    nc.scalar.activation(
                        pTa[:, kc0 * 512 : kc0 * 512 + ew],
                        scA[:, 0:ew], EXP, scale=0.125,
                    )
                    nc.scalar.activation(
                        pTb[:, kc0 * 512 : kc0 * 512 + ew],
                        scB[:, 0:ew], EXP, scale=0.125,
                    )

                def pv_grp(p, g, pTa, pTb, psO_a, psO_b):
                    kc0, glen = GROUPS[g]
                    for u in range(glen):
                        kc = kc0 + u
                        if ablate == "pv_lite" and kc not in (0, NKC - 1):
                            continue
                        nc.tensor.matmul(
                            psO_a[0:65, :],
                            v_blk[kc // 4][:, kc % 4, 2 * p, :],
                            pTa[:, kc * 512 : (kc + 1) * 512],
                            start=(kc == 0) or ablate == "pv_lite",
                            stop=(kc == NKC - 1) or ablate == "pv_lite",
                        )
                        nc.tensor.matmul(
                            psO_b[0:65, :],
                            v_blk[kc // 4][:, kc % 4, 2 * p + 1, :],
                            pTb[:, kc * 512 : (kc + 1) * 512],
                            start=(kc == 0) or ablate == "pv_lite",
                            stop=(kc == NKC - 1) or ablate == "pv_lite",
                        )

                def finalize_pair(j, p, psO_a, psO_b):
                    outT = outt_pool.tile([128, QB], F32, tag="outT")
                    nc.vector.tensor_copy(outT[0:64, :], psO_a[0:64, :])
                    nc.vector.tensor_copy(outT[64:128, :], psO_b[0:64, :])
                    rsum_a = small_pool.tile([1, QB], F32, tag="rsuma")
                    rsum_b = small_pool.tile([1, QB], F32, tag="rsumb")
                    nc.vector.reciprocal(rsum_a[:], psO_a[64:65, :])
                    nc.vector.reciprocal(rsum_b[:], psO_b[64:65, :])
                    ob = ob_pool.tile([128, 4, 128], F32, tag="ob")
                    for tci in range(4):
                        tp = proj_ps.tile([128, 512], F32, tag="proj")
                        nc.tensor.transpose(
                            tp[:, 0:128], outT[:, tci * 128 : (tci + 1) * 128], idn[:]
                        )
                        nc.tensor.transpose(
                            tp[:, 128:129],
                            rsum_a[:, tci * 128 : (tci + 1) * 128],
                            idn[0:1, 0:1],
                        )
                        nc.tensor.transpose(
                            tp[:, 129:130],
                            rsum_b[:, tci * 128 : (tci + 1) * 128],
                            idn[0:1, 0:1],
                        )
                        rT = small_pool.tile([128, 2], F32, tag="rT")
                        nc.vector.tensor_copy(rT[:], tp[:, 128:130])
                        nc.vector.tensor_scalar_mul(
                            ob[:, tci, 0:64], tp[:, 0:64], rT[:, 0:1]
                        )
                        nc.vector.tensor_scalar_mul(
                            ob[:, tci, 64:128], tp[:, 64:128], rT[:, 1:2]
                        )
                    nc.gpsimd.dma_start(
                        out[j * QB : (j + 1) * QB, p * 128 : (p + 1) * 128]
                        .rearrange("(tb t) c -> t tb c", t=128),
                        ob[:],
                    )

                def attention_pair(j, p, qT_blk):
                    # software-pipelined: emit scores one group ahead of PV so
                    # the in-order PE stream never blocks on the current
                    # group's exp (scores g+1 runs while ACT does exp g).
                    pTa = pt_pool.tile([128, NKC * 512], BF16, tag="pt", name=f"pTa_{j}_{p}")
                    pTb = pt_pool.tile([128, NKC * 512], BF16, tag="pt", name=f"pTb_{j}_{p}")
                    psO_a = pv_ps.tile([128, 512], F32, tag="pv", name=f"psOa_{j}_{p}")
                    psO_b = pv_ps.tile([128, 512], F32, tag="pv", name=f"psOb_{j}_{p}")
                    ng = len(GROUPS)
                    scores_grp(j, p, 0, qT_blk, pTa, pTb)
                    for g in range(ng):
                        if g + 1 < ng:
                            scores_grp(j, p, g + 1, qT_blk, pTa, pTb)
                        pv_grp(p, g, pTa, pTb, psO_a, psO_b)
                    finalize_pair(j, p, psO_a, psO_b)

                # --- attention ---
                if prologue_only:
                    dummy = ob_pool.tile([128, 128], F32, tag="ob")
                    nc.vector.tensor_copy(dummy[:], kT_blk[3][:, 3, 0:128])
                    nc.gpsimd.dma_start(out[0:128, 0:128], dummy[:])
                else:
                    for j in range(NQB):
                        for p in range(NPAIR):
                            attention_pair(j, p, qT_all[j])

    return nc


def _shard_inputs(Q, V, K, wq, bq, wk, bk, wv, bv):
    in_maps = []
    for c in range(N_CORES):
        b, half = c // 2, c % 2
        lo = half * COLS
        bq_s, bk_s, bv_s = bq[lo : lo + COLS], bk[lo : lo + COLS], bv[lo : lo + COLS]
        in_maps.append(
            {
                "xq": np.ascontiguousarray(Q[b]),
                "xk": np.ascontiguousarray(K[b]),
                "xv": np.ascontiguousarray(V[b]),
                "wq": np.ascontiguousarray(wq[:, lo : lo + COLS]),
                "wk": np.ascontiguousarray(wk[:, lo : lo + COLS]),
                "wv": np.ascontiguousarray(wv[:, lo : lo + COLS]),
                "bqp": np.ascontiguousarray(bq_s.reshape(NPAIR, 128).T),
                "bkp": np.ascontiguousarray(bk_s.reshape(NPAIR, 128).T),
                "bvb": np.ascontiguousarray(
                    np.broadcast_to(bv_s, (128, COLS))
                ),
            }
        )
    return in_maps


class SpmdRunner:
    """Compile a Bass program once; run it on 8 cores via PJRT with timing.

    Mirrors bass2jax.run_bass_via_pjrt's multi-core path but keeps the jitted
    executable so repeat executions don't re-trace/re-compile.
    """

    def __init__(self, nc: bass.Bass, n_cores: int = 8):
        import jax
        from jax.sharding import Mesh, PartitionSpec
        from jax.experimental.shard_map import shard_map
        from concourse import bass2jax
        from concourse.bass2jax import _bass_exec_p, install_neuronx_cc_hook

        install_neuronx_cc_hook()
        self.nc = nc
        self.n_cores = n_cores
        self._jax = jax
        self._PartitionSpec = PartitionSpec

        in_names, out_names, out_avals, zero_outs = [], [], [], []
        partition_name = (
            nc.partition_id_tensor.name if nc.partition_id_tensor else None
        )
        for alloc in nc.m.functions[0].allocations:
            if not isinstance(alloc, mybir.MemoryLocationSet):
                continue
            name = alloc.memorylocations[0].name
            if alloc.kind == "ExternalInput":
                if name != partition_name:
                    in_names.append(name)
            elif alloc.kind == "ExternalOutput":
                out_names.append(name)
                shape = tuple(alloc.tensor_shape)
                dtype = mybir.dt.np(alloc.dtype)
                out_avals.append(jax.core.ShapedArray(shape, dtype))
                zero_outs.append(np.zeros(shape, dtype))

        self.in_names = in_names
        self.out_names = out_names
        self.out_avals = out_avals
        self.zero_outs = zero_outs
        n_params = len(in_names)
        n_outs = len(out_avals)
        all_in_names = list(in_names) + list(out_names)
        if partition_name is not None:
            all_in_names.append(partition_name)

        donate = tuple(range(n_params, n_params + n_outs))

        def _body(*args):
            operands = list(args)
            if partition_name is not None:
                operands.append(bass2jax.partition_id_tensor())
            outs = _bass_exec_p.bind(
                *operands,
                out_avals=tuple(out_avals),
                in_names=tuple(all_in_names),
                out_names=tuple(out_names),
                lowering_input_output_aliases=(),
                sim_require_finite=True,
                sim_require_nnan=True,
                nc=nc,
            )
            return tuple(outs)

        devices = jax.devices()[:n_cores]
        self.mesh = Mesh(np.asarray(devices), ("core",))
        in_specs = (PartitionSpec("core"),) * (n_params + n_outs)
        out_specs = (PartitionSpec("core"),) * len(out_names)
        self.sharded = jax.jit(
            shard_map(
                _body,
                mesh=self.mesh,
                in_specs=in_specs,
                out_specs=out_specs,
                check_rep=False,
            ),
            donate_argnums=donate,
            keep_unused=True,
        )

    def run(self, in_maps, iters: int = 1):
        """Returns (results_per_core, best_iter_seconds)."""
        import time as _time

        jax = self._jax
        from jax.sharding import NamedSharding

        sh = NamedSharding(self.mesh, self._PartitionSpec("core"))
        per_core = [
            [np.asarray(m[name]) for name in self.in_names] for m in in_maps
        ]
        concat_in = [
            np.concatenate([per_core[c][i] for c in range(self.n_cores)], axis=0)
            for i in range(len(self.in_names))
        ]
        concat_in = [jax.device_put(a, sh) for a in concat_in]
        for a in concat_in:
            a.block_until_ready()
        times = []
        out_arrs = None
        for _ in range(iters):
            concat_zeros = [
                jax.device_put(
                    np.zeros((self.n_cores * z.shape[0], *z.shape[1:]), z.dtype),
                    sh,
                )
                for z in self.zero_outs
            ]
            for z in concat_zeros:
                z.block_until_ready()
            t0 = _time.perf_counter()
            out_arrs = self.sharded(*concat_in, *concat_zeros)
            for o in out_arrs:
                o.block_until_ready()
            t1 = _time.perf_counter()
            times.append(t1 - t0)
        results = [
            {
                name: np.asarray(out_arrs[i]).reshape(
                    self.n_cores, *self.out_avals[i].shape
                )[c]
                for i, name in enumerate(self.out_names)
            }
            for c in range(self.n_cores)
        ]
        return results, min(times)


_RUNNER = None


def _get_runner():
    global _RUNNER
    if _RUNNER is None:
        _RUNNER = SpmdRunner(build_core_program(), n_cores=N_CORES)
    return _RUNNER


def kernel(**inputs) -> np.ndarray:
    inputs = {k: np.asarray(v) for k, v in inputs.items()}
    in_maps = _shard_inputs(**inputs)
    runner = _get_runner()
    results, _ = runner.run(in_maps, iters=1)
    out = np.empty((B, S, D_MODEL), np.float32)
    for c in range(N_CORES):
        b, half = c // 2, c % 2
        out[b, :, half * COLS : (half + 1) * COLS] = results[c]["out"]
    return out

